# revision 42
# baseline (speedup 1.0000x reference)
"""Causal self-attention with RoPE on 8 Trainium2 NeuronCores.

Sharding: Megatron-style head parallelism. 16 heads / 8 cores = 2 heads per
core. Each core computes q/k/v projections for its 2 heads (column-parallel),
full causal attention for those heads, and a partial output projection
(row-parallel slice of w_o). The host sums the 8 partial outputs.

Optimizations vs the f32r baseline (952us -> ~730us):
- All matmul operands and all HBM traffic are bf16 (fp32 PSUM accumulate).
  Halves DMA bytes and SBUF read pressure; rel-err ~0.6% << the 2e-2 gate.
  (fp8 / DoubleRow was measured to give ZERO matmul speedup on this
  hardware - every dtype streams 1 moving row per cycle.)
- Softmax denominators accumulate via an all-ones [128,128] lhsT, so the
  per-q sums land already replicated across all 128 partitions: no
  broadcast matmul, no PSUM->SBUF staging. 1/den = exp(-ln(den)) via two
  ACT table ops (custom-DVE reciprocal ops fail this toolchain's codegen;
  plain DVE reciprocal is 13x slower than a multiply).
- Causal diagonal: kv tile dg of each TQ-wide q-group is processed with a
  single score/PV/den matmul over the contiguous live q-range
  [dg*TK, TQ); the exact-diagonal leading TK columns are masked
  multiplicatively on the DVE after exp. Saves ~25% of attention rows AND
  keeps the instruction count low (~20-30ns fixed cost per PE matmul).
- Software pipelining by emission order: PV/den of kv tile i are emitted
  after the score of tile i+1 (exp hides under the next score);
  normalization is deferred one q-group and the output projection two
  q-groups (as a generator interleaved into the attention stream), both
  carried across batch boundaries.
- DMA issue costs ~0.6us per dma_start on the issuing queue: weights +
  constants issue from the scalar queue in parallel with x tiles on the
  sync queue; x tiles use 2-dc-chunk transfers, y rows one wide staging
  tile + 2 half-row DMAs; the next batch's x tiles are prefetched before
  the attention phase begins.

On-chip layout: everything transposed. Host passes xT = x^T per batch
[B, D, T]; projections produce qT/kT [dh, t] directly and v [t, dh]
(lhsT = xT chunk, rhs = w_v slice). Scores are computed transposed,
ST[kv, q] = matmul(lhsT=kT_chunk, rhs=qT_group), which makes P^T directly
usable as the moving operand of the PV matmul - no on-chip transposes.
No max-subtraction: logits are q.k/sqrt(dh) with unit-ish variance,
|logit| < ~8 << 88 (fp32 exp overflow), identical math to the
max-subtracted reference. The attention scale 1/sqrt(dh) is folded into
w_q on the host.
"""

import numpy as np

B, T, D = 4, 2048, 2048
H, DH = 16, 128
NCORES = 8
HPC = H // NCORES  # heads per core
THETA = 10000.0

TT = 512  # projection t-tile (moving dim of q/k projection matmuls)
TQ = 512  # attention q-group width
TK = 128  # kv tile (contraction chunk of PV / partition dim of ST)


def _rope_tables(seq_len, d_head, theta):
    # Matches reference.rope_cos_sin numerics, then transposes to [dh, t]
    # and folds the rotate-half sign into sin.
    inv_freq = 1.0 / (theta ** (np.arange(0, d_head, 2, dtype=np.float32) / d_head))
    t = np.arange(seq_len, dtype=np.float32)
    freqs = np.einsum("i,j->ij", t, inv_freq)
    emb = np.concatenate([freqs, freqs], axis=-1)  # [T, dh]
    cosT = np.ascontiguousarray(np.cos(emb).astype(np.float32).T)  # [dh, T]
    sinT = np.ascontiguousarray(np.sin(emb).astype(np.float32).T)
    sgn = np.ones((d_head, 1), np.float32)
    sgn[: d_head // 2] = -1.0
    return cosT, sinT * sgn


def _legalize_waits(nc, mybir):
    """Walrus on this toolchain refuses more than one embedded sync wait
    per engine instruction. Hoist extra waits into standalone
    EventSemaphore instructions on the same engine queue (the sequencer
    executes them in-stream before the instruction, same gating)."""
    n = 0
    for f in nc.m.functions:
        for bb in f.blocks:
            out = []
            for inst in bb.instructions:
                si = inst.sync_info
                if (si and si.on_wait and len(si.on_wait) > 1
                        and not isinstance(inst, mybir.InstEventSemaphore)):
                    for w in si.on_wait[:-1]:
                        out.append(mybir.InstEventSemaphore(
                            name=f"WH-{n}", engine=inst.engine,
                            sync_info=mybir.SyncInfo(
                                on_wait=[w], on_update=[])))
                        n += 1
                    inst.sync_info = mybir.SyncInfo(
                        on_wait=[si.on_wait[-1]],
                        on_update=list(si.on_update))
                out.append(inst)
            bb.instructions = out
    return n


def _build_nc(b_sz, t_sz, d_sz, legalize=True):
    import concourse.bass as bass
    import concourse.tile as tile
    from concourse import mybir

    f32 = mybir.dt.float32
    bf16 = mybir.dt.bfloat16
    EXP = mybir.ActivationFunctionType.Exp
    LN = mybir.ActivationFunctionType.Ln

    DC = d_sz // 128         # contraction chunks
    NQG = t_sz // TQ         # q groups per (batch, head)
    NKT = t_sz // TK         # kv tiles
    KPG = TQ // TK           # kv tiles per q group (diagonal span)

    nc = bass.Bass("TRN2", target_bir_lowering=False, debug=False,
                   enable_asserts=False, dynamic_dma_scratch_size=2048)

    NW = HPC * DH
    xT = nc.dram_tensor("xT", [b_sz, d_sz, t_sz], bf16, kind="ExternalInput")
    wq = nc.dram_tensor("wq", [d_sz, NW], bf16, kind="ExternalInput")
    wk = nc.dram_tensor("wk", [d_sz, NW], bf16, kind="ExternalInput")
    wv = nc.dram_tensor("wv", [d_sz, NW], bf16, kind="ExternalInput")
    wo = nc.dram_tensor("wo", [HPC * DH, d_sz], bf16, kind="ExternalInput")
    cos = nc.dram_tensor("cos", [DH, t_sz], f32, kind="ExternalInput")
    sin = nc.dram_tensor("sin", [DH, t_sz], f32, kind="ExternalInput")
    tri = nc.dram_tensor("tri", [TK, TK], bf16, kind="ExternalInput")
    one = nc.dram_tensor("one", [128, 128], bf16, kind="ExternalInput")
    y = nc.dram_tensor("y", [b_sz, t_sz, d_sz], bf16, kind="ExternalOutput")

    xT_r = xT.ap().rearrange("b (dc p) t -> b p dc t", p=128)
    wq_r = wq.ap().rearrange("(dc p) n -> p dc n", p=128)
    wk_r = wk.ap().rearrange("(dc p) n -> p dc n", p=128)
    wv_r = wv.ap().rearrange("(dc p) n -> p dc n", p=128)
    wo_r = wo.ap().rearrange("(h p) n -> p h n", p=128)
    y_r = y.ap()

    with tile.TileContext(nc) as tc:
        with (
            tc.tile_pool(name="consts", bufs=1) as consts,
            tc.tile_pool(name="wpool", bufs=1) as wpool,
            tc.tile_pool(name="qkv", bufs=1) as qkv,
            tc.tile_pool(name="xpool", bufs=4) as xpool,
            tc.tile_pool(name="rope", bufs=3) as rope,
            tc.tile_pool(name="pex", bufs=4) as pexp,
            tc.tile_pool(name="sax", bufs=2) as sax,
            tc.tile_pool(name="otn", bufs=10) as otnp,
            tc.tile_pool(name="psS", bufs=2, space="PSUM") as psS,
            tc.tile_pool(name="psO", bufs=2, space="PSUM") as psO,
            tc.tile_pool(name="psR", bufs=2, space="PSUM") as psR,
            tc.tile_pool(name="psY", bufs=2, space="PSUM") as psY,
        ):
            cos_sb = consts.tile([DH, t_sz], f32)
            sin_sb = consts.tile([DH, t_sz], f32)
            tri_sb = consts.tile([TK, TK], bf16)
            ones_sb = consts.tile([128, 128], bf16)

            wq_sb = wpool.tile([128, DC, NW], bf16)
            wk_sb = wpool.tile([128, DC, NW], bf16)
            wv_sb = wpool.tile([128, DC, NW], bf16)
            wo_sb = wpool.tile([128, HPC, d_sz], bf16)

            # DMA issue runs on two engine queues in parallel: the sync
            # engine paces the x-tile / y streams, the scalar engine
            # issues weights + constants, so the cold start is not
            # serialized on one sequencer's ~0.6us per descriptor.
            # (gpsimd dma_start hangs the device on this runtime.)
            xt_first = xpool.tile([128, DC, TT], bf16, tag="xt",
                                  name="xt_first")
            # cos/sin of the first tile gate the very first RoPE op (which
            # in turn gates PSUM pp-buffer recycling) - they go FIRST on
            # the scalar queue. wq/wk interleave per chunk so the k
            # projections never wait on a second full-weight-tensor issue.
            nc.scalar.dma_start(cos_sb[:, 0:TT], cos.ap()[:, 0:TT])
            nc.scalar.dma_start(sin_sb[:, 0:TT], sin.ap()[:, 0:TT])
            for dc in range(DC):
                nc.sync.dma_start(xt_first[:, dc, :], xT_r[0, :, dc, 0:TT])
                nc.scalar.dma_start(wq_sb[:, dc, :], wq_r[:, dc, :])
                nc.scalar.dma_start(wk_sb[:, dc, :], wk_r[:, dc, :])
                nc.scalar.dma_start(wv_sb[:, dc, :], wv_r[:, dc, :])

            def load_consts():
                # everything here is first needed in the attention phase
                # (t > ~90us): emitted after tile 1's x DMAs
                for i in range(1, t_sz // TT):
                    sl = slice(i * TT, (i + 1) * TT)
                    nc.scalar.dma_start(cos_sb[:, sl], cos.ap()[:, sl])
                    nc.scalar.dma_start(sin_sb[:, sl], sin.ap()[:, sl])
                nc.scalar.dma_start(tri_sb[:], tri.ap())
                nc.scalar.dma_start(ones_sb[:], one.ap())
                for hh in range(HPC):
                    for nch in range(d_sz // 512):
                        nsl = slice(nch * 512, (nch + 1) * 512)
                        nc.scalar.dma_start(wo_sb[:, hh, nsl],
                                            wo_r[:, hh, nsl])

            def emit_xt(b, tt):
                xt = xpool.tile([128, DC, TT], bf16, tag="xt", name="xt")
                tsl = slice(tt * TT, (tt + 1) * TT)
                for dc in range(0, DC, 2):
                    nc.sync.dma_start(xt[:, dc:dc + 2, :],
                                      xT_r[b, :, dc:dc + 2, tsl])
                return xt

            # deferred-normalization / output-projection closures carry
            # across batch boundaries so no per-batch pipeline drain
            otn_tiles = {}
            pending1 = []
            pending2 = []
            nxt_tiles = None
            for b in range(b_sz):
                # ---------------- phase A: projections + RoPE ----------
                qT = [qkv.tile([DH, t_sz], bf16, tag=f"qT{h}", name=f"qT{h}")
                      for h in range(HPC)]
                kT = [qkv.tile([DH, t_sz], bf16, tag=f"kT{h}", name=f"kT{h}")
                      for h in range(HPC)]
                vv = qkv.tile([128, NKT, HPC * DH], bf16, tag="vv", name="vv")

                for tt in range(t_sz // TT):
                    tsl = slice(tt * TT, (tt + 1) * TT)
                    if b == 0:
                        xt = xt_first if tt == 0 else emit_xt(b, tt)
                        if tt == 1:
                            load_consts()
                    else:
                        # DMAs were emitted before the previous batch's
                        # attention phase - the transfers landed long ago
                        xt = nxt_tiles[tt]

                    for h in range(HPC):
                        hs = slice(h * DH, (h + 1) * DH)
                        for dst, w_sb in ((qT[h], wq_sb), (kT[h], wk_sb)):
                            pp = psS.tile([128, TT], f32, tag="st")
                            for dc in range(DC):
                                nc.tensor.matmul(
                                    pp[:],
                                    w_sb[:, dc, hs],
                                    xt[:, dc, :],
                                    start=(dc == 0), stop=(dc == DC - 1),
                                )
                            # RoPE: dst = pp*cos + swap(pp)*sin_signed
                            sh = rope.tile([DH, TT], f32, tag="sh")
                            nc.vector.tensor_mul(
                                sh[0:64, :], pp[64:128, :], sin_sb[0:64, tsl])
                            nc.vector.tensor_mul(
                                sh[64:128, :], pp[0:64, :], sin_sb[64:128, tsl])
                            tmp = rope.tile([DH, TT], f32, tag="tmp")
                            nc.vector.tensor_mul(tmp[:], pp[:], cos_sb[:, tsl])
                            nc.vector.tensor_add(dst[:, tsl], tmp[:], sh[:])

                    for ts2 in range(TT // TK):
                        ts3 = slice(ts2 * TK, (ts2 + 1) * TK)
                        vp = psS.tile([128, TT], f32, tag="st")
                        for dc in range(DC):
                            nc.tensor.matmul(
                                vp[:, 0:HPC * DH],
                                xt[:, dc, ts3],
                                wv_sb[:, dc, :],
                                start=(dc == 0), stop=(dc == DC - 1),
                            )
                        kv_i = tt * (TT // TK) + ts2
                        nc.scalar.copy(vv[:, kv_i, :], vp[:, 0:HPC * DH])

                # prefetch the whole next batch's x tiles now: the xpool
                # ring frees as this batch's projections complete, and the
                # transfers hide under ~100us of attention
                if b + 1 < b_sz:
                    nxt_tiles = [emit_xt(b + 1, tt)
                                 for tt in range(t_sz // TT)]

                # ---------------- phase B + C: attention + out proj ----
                for h in range(HPC):
                    hs = slice(h * DH, (h + 1) * DH)
                    for qi in range(NQG):
                        # deferred work first: the previous group's
                        # normalization, and the output projection from two
                        # groups back as a generator interleaved with this
                        # group's attention stream (outproj matmuls fill
                        # exp-wait gaps; attention matmuls cover the
                        # PSUM->bf16 staging-copy drain)
                        if pending1:
                            pending1.pop(0)()
                        active_gen = None
                        if len(pending2) > 1:
                            fac = pending2.pop(0)
                            if fac is not None:
                                active_gen = fac()

                        outp = psO.tile([DH, TQ], f32, tag="outT")
                        denp = psR.tile([DH, TQ], f32, tag="den")

                        def qk_exp(ki, q0, n, masked):
                            # score matmul [TK, n] + exp (+ causal mask on
                            # the leading TK columns = the exact-diagonal
                            # tile, zeroed multiplicatively after exp)
                            stp = psS.tile([128, TT], f32, tag="st")
                            nc.tensor.matmul(
                                stp[:, 0:n],
                                kT[h][:, ki * TK:(ki + 1) * TK],
                                qT[h][:, q0:q0 + n],
                                start=True, stop=True,
                            )
                            pex = pexp.tile([TK, TQ], bf16, tag="pex",
                                            name="pex")
                            nc.scalar.activation(pex[:, 0:n], stp[:, 0:n],
                                                 EXP)
                            if masked:
                                nc.vector.tensor_mul(
                                    pex[:, 0:TK], pex[:, 0:TK], tri_sb[:])
                            return pex

                        # kv tiles of this q group: off-diagonal full-width
                        # tiles, then the diagonal tiles, each covering the
                        # causally live q-range [dg*TK, TQ). The PV/den
                        # matmuls for tile i are emitted AFTER the score of
                        # tile i+1, so each exp hides under the next score
                        # and the PE never waits on the ACT engine.
                        items = [(ki, 0, TQ, False, ki == 0, False)
                                 for ki in range(qi * KPG)]
                        items += [(qi * KPG + dg, dg * TK, TQ - dg * TK,
                                   True, qi == 0 and dg == 0, dg == KPG - 1)
                                  for dg in range(KPG)]

                        def emit_pv(it, pex):
                            ki, c0, n, _, st_f, sp_f = it
                            dsl = slice(c0, c0 + n)
                            nc.tensor.matmul(
                                outp[:, dsl], vv[:, ki, hs], pex[:, 0:n],
                                start=st_f, stop=sp_f,
                                skip_group_check=True,
                            )
                            nc.tensor.matmul(
                                denp[:, dsl], ones_sb[:], pex[:, 0:n],
                                start=st_f, stop=sp_f,
                                skip_group_check=True,
                            )

                        pend = None
                        for it in items:
                            pex = qk_exp(it[0], qi * TQ + it[1], it[2],
                                         it[3])
                            if active_gen is not None:
                                next(active_gen, None)
                                if next(active_gen, None) is None:
                                    active_gen = None
                            if pend is not None:
                                emit_pv(*pend)
                            pend = (it, pex)
                        emit_pv(*pend)
                        if active_gen is not None:
                            for _ in active_gen:
                                pass

                        def stage1(h=h, qi=qi, outp=outp, denp=denp):
                            # deferred one q-group: 1/den computed as
                            # exp(-ln(den)) on the ACT engine: two table ops
                            # (~1e-3 rel err, fine for a softmax
                            # denominator) instead of the 13x-slower DVE
                            # reciprocal; the normalization multiply reads
                            # the PV PSUM tile directly.
                            lnt = sax.tile([DH, TQ], f32, tag="lnt",
                                           name="lnt")
                            nc.scalar.activation(lnt[:], denp[:], LN)
                            rcp = sax.tile([DH, TQ], f32, tag="rcp",
                                           name="rcp")
                            nc.scalar.activation(rcp[:], lnt[:], EXP,
                                                 scale=-1.0)
                            otn = otnp.tile([DH, TQ], bf16, tag="otn",
                                            name="otn")
                            nc.vector.tensor_mul(otn[:], outp[:], rcp[:])
                            otn_tiles[(h, qi)] = otn

                        def stage2(qi=qi, b=b):
                            # generator, consumed two q-groups later: by
                            # then the otn tiles of both heads exist and
                            # their DVE writes have had a full group to
                            # drain
                            for tc2 in range(TQ // TK):
                                tq0 = qi * TQ + tc2 * TK
                                # one wide staging tile per 128-token row
                                # block: 2 half-row DMAs (2KB descriptors)
                                # instead of 4, keeping the sync queue free
                                # to prefetch the next batch's x tiles
                                ysb = pexp.tile([TK, d_sz], bf16, tag="ysb",
                                                bufs=4, name="ysb")
                                for nch in range(d_sz // 512):
                                    yp = psY.tile([TK, 512], f32, tag="y",
                                                  name="yp")
                                    for hh in range(HPC):
                                        nc.tensor.matmul(
                                            yp[:],
                                            otn_tiles[(hh, qi)][
                                                :, tc2 * TK:(tc2 + 1) * TK],
                                            wo_sb[:, hh,
                                                  nch * 512:(nch + 1) * 512],
                                            start=(hh == 0),
                                            stop=(hh == HPC - 1),
                                        )
                                    # DVE only: the ACT engine stays free
                                    # for the latency-critical softmax exps
                                    nc.vector.tensor_copy(
                                        ysb[:, nch * 512:(nch + 1) * 512],
                                        yp[:])
                                    yield True
                                for half in range(2):
                                    hsl = slice(half * (d_sz // 2),
                                                (half + 1) * (d_sz // 2))
                                    nc.sync.dma_start(
                                        y_r[b, tq0:tq0 + TK, hsl],
                                        ysb[:, hsl])
                                yield True

                        pending1.append(stage1)
                        pending2.append(stage2 if h == HPC - 1 else None)
            for fn in pending1:
                fn()
            for fac in pending2:
                if fac is not None:
                    for _ in fac():
                        pass
    if legalize:
        _legalize_waits(nc, mybir)
    return nc


_NC_CACHE = {}
LAST_RESULT = None


def _get_nc(b_sz, t_sz, d_sz):
    key = (b_sz, t_sz, d_sz)
    if key not in _NC_CACHE:
        _NC_CACHE[key] = _build_nc(b_sz, t_sz, d_sz)
    return _NC_CACHE[key]


def kernel(x, w_q, w_k, w_v, w_o):
    import ml_dtypes
    from concourse.bass_utils import run_bass_kernel_spmd

    bf16 = ml_dtypes.bfloat16
    b_sz, t_sz, d_sz = x.shape
    scale = np.float32(1.0 / np.sqrt(DH))

    xT = np.ascontiguousarray(
        np.asarray(x, np.float32).transpose(0, 2, 1)).astype(bf16)
    w_q = np.asarray(w_q, np.float32)
    w_k = np.asarray(w_k, np.float32)
    w_v = np.asarray(w_v, np.float32)
    w_o = np.asarray(w_o, np.float32)
    cosT, sinT = _rope_tables(t_sz, DH, THETA)
    r = np.arange(TK)
    tri01 = (r[None, :] >= r[:, None]).astype(bf16)  # [kv, q]: keep q >= kv

    in_maps = []
    for c in range(NCORES):
        cs = slice(c * HPC * DH, (c + 1) * HPC * DH)
        in_maps.append({
            "xT": xT,
            "wq": np.ascontiguousarray(w_q[:, cs] * scale).astype(bf16),
            "wk": np.ascontiguousarray(w_k[:, cs]).astype(bf16),
            "wv": np.ascontiguousarray(w_v[:, cs]).astype(bf16),
            "wo": np.ascontiguousarray(w_o[cs, :]).astype(bf16),
            "cos": cosT,
            "sin": sinT,
            "tri": tri01,
            "one": np.ones((128, 128), bf16),
        })

    nc = _get_nc(b_sz, t_sz, d_sz)
    res = run_bass_kernel_spmd(nc, in_maps, core_ids=list(range(NCORES)))
    global LAST_RESULT
    LAST_RESULT = res

    out = res.results[0]["y"].astype(np.float32)
    for c in range(1, NCORES):
        out += res.results[c]["y"].astype(np.float32)
    return out
